# revision 10
# baseline (speedup 1.0000x reference)
"""ObjectDecoder kernel for Trainium2 (8 NeuronCores, data-parallel over batch).

Computes out[b, o, a, p, k] = sum_d x[b, o, d] * W[o, a, p, d, k] + bias[o, a, p, k]
  x: [16384, 16, 256] f32, W: [16, 4, 2, 256, 8] f32, b: [16, 4, 2, 8] f32
  out: [16384, 16, 4, 2, 8] f32

Per-core plan (batch shard of 2048 rows), bf16 data path (tolerance 2e-2 vs
bf16 error ~3e-3): halves HBM traffic and runs the PE at 1 cycle/row.

The kernel is wire-bound on the 16 shared DMA engines (~26 B/ns each, ~420
GB/s aggregate). Measured TRN2 behaviors this schedule works around:
  - The biggest x load rides first; W/bias follow it in-ring (they complete
    by the time that load's matmuls can start, and on any other queue they
    starve behind the x stream for ~13us, stalling the first matmul).
  - One slow DMA engine (also carrying misc queues) lags the others and every
    DMA completion semaphore waits on it; its deficit is ~26% on 32KB
    descriptors but only ~9% on 8KB ones, so all large loads are padded in
    DRAM to force 8KB descriptors, shrinking the end-of-kernel semaphore lag.
  - The Tile framework rotates ~8 DMA-completion semaphores; a reused
    semaphore adds a wait for the 8-back DMA's completion. With exactly 8
    loads ahead of the 7 stores, every store's reuse guard lands on an early
    load it transitively depends on anyway.
  - Each of the 4 output tiles is a distinct buffer so the ACT chain never
    waits on a store's completion semaphore.
  - The last pair is loaded in batch-halves and stored per-chunk on the
    (by then idle) sync queue so the drain tail is short.

Layouts (host-prepped):
  quads  [p(128), obj(4), 4096+pad] with element g = k*2048 + b, d = k*128+p
  xp6    [p(128), obj(2), 4096+pad]
  halves [p(128), obj(2), k(2), b(1024)]  (contiguous)
  wt[d_lo(128), k_chunk(2), obj(16), apk(64)], bt[(o2*64+apk)(128), pair(8)]
  out[group(4), (o2*64+apk)(128), pair_in_group(2), b]  (obj = 4g + 2pr + o2)

Per pair of objects: 4 matmuls [K=128, M=64, N=512] accumulate into a
[128, 512] PSUM bank (objects 2i / 2i+1 stacked on partitions); the scalar
engine evacuates PSUM with a fused per-partition bias add, converting to bf16.
"""

import os
from contextlib import ExitStack

os.environ.setdefault("JAX_PLATFORMS", "axon")

import ml_dtypes
import numpy as np

import concourse.bass as bass
import concourse.mybir as mybir
import concourse.tile as tile
from concourse import bacc
from concourse.bass_utils import run_bass_kernel_spmd

B, N_OBJ, DIM_IN, APK = 16384, 16, 256, 64
N_CORES = 8
BS = B // N_CORES          # 2048 batch rows per core
NT = 512                   # moving-operand tile (one PSUM bank in fp32)
NB = BS // NT              # 4 batch chunks per core
F32 = mybir.dt.float32
BF16 = mybir.dt.bfloat16
NP_BF16 = ml_dtypes.bfloat16
OBJ_ELEMS = 2 * BS         # per-(partition, object) flat elements: k*2048 + b
PAD = 256                  # pad per object line -> forces 8KB descriptors

# x segments: (name, [objects], batch range, padded)
SEGS = [
    ("xq0", [0, 1, 2, 3], (0, BS), True),
    ("xq1", [4, 5, 6, 7], (0, BS), True),
    ("xq2", [8, 9, 10, 11], (0, BS), True),
    ("xp6", [12, 13], (0, BS), True),
    ("xh0", [14, 15], (0, BS // 2), False),
    ("xh1", [14, 15], (BS // 2, BS), False),
]

_CACHE: dict = {}


def _build_nc():
    nc = bacc.Bacc("TRN2", target_bir_lowering=False, debug=False)

    xs = {}
    for name, objs, br, padded in SEGS:
        if padded:
            shape = [128, len(objs), OBJ_ELEMS + PAD]
        else:
            shape = [128, len(objs), 2, br[1] - br[0]]
        xs[name] = nc.declare_dram_parameter(name, shape, BF16, isOutput=False)
    wt = nc.declare_dram_parameter("wt", [128, 2, N_OBJ, APK], BF16, isOutput=False)
    bt = nc.declare_dram_parameter("bt", [128, N_OBJ // 2], F32, isOutput=False)
    out = nc.declare_dram_parameter("out", [4, 128, 2, BS], BF16, isOutput=True)

    with tile.TileContext(nc) as tc, ExitStack() as ctx:
        wpool = ctx.enter_context(tc.tile_pool(name="w", bufs=1))
        xpools = {
            name: ctx.enter_context(tc.tile_pool(name=name, bufs=1))
            for name, _, _, _ in SEGS
        }
        psum = ctx.enter_context(
            tc.tile_pool(name="ps", bufs=8, space=bass.MemorySpace.PSUM)
        )
        opool = ctx.enter_context(tc.tile_pool(name="o", bufs=1))

        w_sb = wpool.tile([128, 2, N_OBJ, APK], BF16)
        b_sb = wpool.tile([128, N_OBJ // 2], F32)

        # Ring order: xq0 streams first; W/b complete in-ring long before
        # xq0's matmuls can start, without delaying the stream head.
        xt = {}
        for i, (name, objs, br, padded) in enumerate(SEGS):
            if padded:
                t = xpools[name].tile(
                    [128, len(objs), OBJ_ELEMS], BF16, name=f"t_{name}"
                )
                nc.sync.dma_start(t[:], xs[name][:, :, :OBJ_ELEMS])
            else:
                t = xpools[name].tile(
                    [128, len(objs), 2, br[1] - br[0]], BF16, name=f"t_{name}"
                )
                nc.sync.dma_start(t[:], xs[name][:])
            xt[name] = t
            if i == 0:
                nc.sync.dma_start(w_sb[:], wt[:])
                nc.sync.dma_start(b_sb[:], bt[:])

        # pair index -> (segment name, first-object index within segment)
        pair_seg = {0: ("xq0", 0), 1: ("xq0", 2), 2: ("xq1", 0), 3: ("xq1", 2),
                    4: ("xq2", 0), 5: ("xq2", 2), 6: ("xp6", 0)}

        ot = {}
        for op in range(7):  # pairs 0-6: full-batch compute
            g, pr = divmod(op, 2)
            if pr == 0:
                ot[g] = opool.tile([128, 2, BS], BF16, name=f"ot{g}")
            seg, j0 = pair_seg[op]
            for n in range(NB):
                ps = psum.tile([128, NT], F32)
                for o2 in range(2):
                    for k in range(2):
                        nc.tensor.matmul(
                            ps[o2 * 64 : (o2 + 1) * 64, :],
                            w_sb[:, k, 2 * op + o2, :],
                            xt[seg][
                                :, j0 + o2, k * BS + n * NT : k * BS + (n + 1) * NT
                            ],
                            start=(k == 0),
                            stop=(k == 1),
                        )
                nc.scalar.activation(
                    ot[g][:, pr, n * NT : (n + 1) * NT],
                    ps[:],
                    mybir.ActivationFunctionType.Identity,
                    bias=b_sb[:, op : op + 1],
                )
            if pr == 1:
                nc.scalar.dma_start(out[g], ot[g])
            elif op == 6:
                nc.scalar.dma_start(out[3, :, 0, :], ot[3][:, 0, :])

        # pair 7: per-chunk compute from the batch-half tiles; stores on the
        # (now idle) sync queue so the scalar ACT chain isn't extended.
        for n in range(NB):
            seg = "xh0" if n < 2 else "xh1"
            b0 = (n % 2) * NT
            ps = psum.tile([128, NT], F32)
            for o2 in range(2):
                for k in range(2):
                    nc.tensor.matmul(
                        ps[o2 * 64 : (o2 + 1) * 64, :],
                        w_sb[:, k, 14 + o2, :],
                        xt[seg][:, o2, k, b0 : b0 + NT],
                        start=(k == 0),
                        stop=(k == 1),
                    )
            nc.scalar.activation(
                ot[3][:, 1, n * NT : (n + 1) * NT],
                ps[:],
                mybir.ActivationFunctionType.Identity,
                bias=b_sb[:, 7:8],
            )
            nc.sync.dma_start(
                out[3, :, 1, n * NT : (n + 1) * NT],
                ot[3][:, 1, n * NT : (n + 1) * NT],
            )

    nc.compile()
    return nc


def _get_nc():
    if "nc" not in _CACHE:
        _CACHE["nc"] = _build_nc()
    return _CACHE["nc"]


def _prep_inputs(x, W, b):
    x16 = np.asarray(x, dtype=np.float32).astype(NP_BF16)
    # wt[d_lo, k_chunk, o, apk]: W[o,a,p,d,k] -> [d,o,apk] -> [2,128,o,apk] -> [128,2,o,apk]
    wt = np.ascontiguousarray(
        np.asarray(W, dtype=np.float32)
        .astype(NP_BF16)
        .transpose(3, 0, 1, 2, 4)
        .reshape(2, 128, N_OBJ, APK)
        .transpose(1, 0, 2, 3)
    )
    # bt[o2*64+apk, pair]
    bt = np.ascontiguousarray(
        np.asarray(b, dtype=np.float32)
        .reshape(N_OBJ // 2, 2, APK)
        .transpose(1, 2, 0)
        .reshape(128, N_OBJ // 2)
    )
    in_maps = []
    for c in range(N_CORES):
        xc = x16[c * BS : (c + 1) * BS]  # [BS, 16, 256] bf16
        m = {"wt": wt, "bt": bt}
        for name, objs, (b0, b1), padded in SEGS:
            # a[p, j, k, b] = xc[b0+b, objs[j], k*128 + p]
            a = xc[b0:b1, objs, :]                       # [nb, no, 256]
            a = a.transpose(2, 1, 0)                     # [256, no, nb]
            a = a.reshape(2, 128, len(objs), b1 - b0)    # [k, p, j, b]
            a = np.ascontiguousarray(a.transpose(1, 2, 0, 3))  # [p, j, k, b]
            if padded:
                no = len(objs)
                buf = np.zeros((128, no, OBJ_ELEMS + PAD), dtype=NP_BF16)
                buf[:, :, :OBJ_ELEMS] = a.reshape(128, no, OBJ_ELEMS)
                m[name] = buf
            else:
                m[name] = a
        in_maps.append(m)
    return in_maps


def kernel(x, W, b, _trace=False, **run_kwargs):
    nc = _get_nc()
    in_maps = _prep_inputs(x, W, b)
    res = run_bass_kernel_spmd(
        nc, in_maps, core_ids=list(range(N_CORES)), trace=_trace, **run_kwargs
    )
    _CACHE["last_results"] = res
    out = np.empty((B, N_OBJ, APK), dtype=np.float32)
    for c in range(N_CORES):
        # arr[g, o2*64+apk, pr, b] -> [b, 4g+2pr+o2, apk]
        a = res.results[c]["out"].astype(np.float32)
        a = a.reshape(4, 2, APK, 2, BS).transpose(4, 0, 3, 1, 2)
        out[c * BS : (c + 1) * BS] = a.reshape(BS, N_OBJ, APK)
    return out.reshape(B, N_OBJ, 4, 2, 8)


# revision 11
# speedup vs baseline: 1.0234x; 1.0234x over previous
"""ObjectDecoder kernel for Trainium2 (8 NeuronCores, data-parallel over batch).

Computes out[b, o, a, p, k] = sum_d x[b, o, d] * W[o, a, p, d, k] + bias[o, a, p, k]
  x: [16384, 16, 256] f32, W: [16, 4, 2, 256, 8] f32, b: [16, 4, 2, 8] f32
  out: [16384, 16, 4, 2, 8] f32

Per-core plan (batch shard of 2048 rows), bf16 data path (tolerance 2e-2 vs
bf16 error ~3e-3): halves HBM traffic and runs the PE at 1 cycle/row.

The kernel is wire-bound on the 16 shared DMA engines (~26 B/ns each, ~420
GB/s aggregate). Measured TRN2 behaviors this schedule works around:
  - W/bias ride FIRST on the sync ring: on any other queue they starve behind
    the x stream (engines round-robin rings per descriptor) and delay the
    first matmul by ~10us. In-ring FIFO lands them before any x data.
  - The Tile framework rotates ~8 DMA-completion semaphores; a reused
    semaphore adds a wait for the 8-back DMA's completion. With exactly 8
    loads (W, b, 6 x segments) ahead of the 7 stores, every store's reuse
    guard lands on an early load it transitively depends on anyway.
  - Each of the 4 output tiles is a distinct buffer so the ACT chain never
    waits on a store's (lagging) completion semaphore.
  - The last pair is loaded in batch-halves and stored per-chunk on the
    (by then idle) sync queue so the drain tail is short.

Layouts (host-prepped):
  xsN[p(128), obj_in_seg, k(2), b] with d = k*128 + p  (contraction on
    partitions; 16-32 KiB contiguous per partition line per load)
  wt[d_lo(128), k_chunk(2), obj(16), apk(64)], bt[(o2*64+apk)(128), pair(8)]
  out[group(4), (o2*64+apk)(128), pair_in_group(2), b]  (obj = 4g + 2pr + o2)

Per pair of objects: 4 matmuls [K=128, M=64, N=512] accumulate into a
[128, 512] PSUM bank (objects 2i / 2i+1 stacked on partitions); the scalar
engine evacuates PSUM with a fused per-partition bias add, converting to bf16.
"""

import os
from contextlib import ExitStack

os.environ.setdefault("JAX_PLATFORMS", "axon")

import ml_dtypes
import numpy as np

import concourse.bass as bass
import concourse.mybir as mybir
import concourse.tile as tile
from concourse import bacc
from concourse.bass_utils import run_bass_kernel_spmd

B, N_OBJ, DIM_IN, APK = 16384, 16, 256, 64
N_CORES = 8
BS = B // N_CORES          # 2048 batch rows per core
NT = 512                   # moving-operand tile (one PSUM bank in fp32)
NB = BS // NT              # 4 batch chunks per core
F32 = mybir.dt.float32
BF16 = mybir.dt.bfloat16
NP_BF16 = ml_dtypes.bfloat16

# x segments: (name, [objects], batch range) — quads first, then pair 6, then
# pair 7 in batch-halves so the drain tail is only half a pair deep.
SEGS = [
    ("xq0", [0, 1, 2, 3], (0, BS)),
    ("xq1", [4, 5, 6, 7], (0, BS)),
    ("xq2", [8, 9, 10, 11], (0, BS)),
    ("xp6", [12, 13], (0, BS)),
    ("xh0", [14, 15], (0, BS // 2)),
    ("xh1", [14, 15], (BS // 2, BS)),
]

_CACHE: dict = {}


def _build_nc():
    nc = bacc.Bacc("TRN2", target_bir_lowering=False, debug=False)

    xs = {
        name: nc.declare_dram_parameter(
            name, [128, len(objs), 2, br[1] - br[0]], BF16, isOutput=False
        )
        for name, objs, br in SEGS
    }
    wt = nc.declare_dram_parameter("wt", [128, 2, N_OBJ, APK], BF16, isOutput=False)
    bt = nc.declare_dram_parameter("bt", [128, N_OBJ // 2], F32, isOutput=False)
    out = nc.declare_dram_parameter("out", [4, 128, 2, BS], BF16, isOutput=True)

    with tile.TileContext(nc) as tc, ExitStack() as ctx:
        wpool = ctx.enter_context(tc.tile_pool(name="w", bufs=1))
        xpools = {
            name: ctx.enter_context(tc.tile_pool(name=name, bufs=1))
            for name, _, _ in SEGS
        }
        psum = ctx.enter_context(
            tc.tile_pool(name="ps", bufs=8, space=bass.MemorySpace.PSUM)
        )
        opool = ctx.enter_context(tc.tile_pool(name="o", bufs=1))

        w_sb = wpool.tile([128, 2, N_OBJ, APK], BF16)
        b_sb = wpool.tile([128, N_OBJ // 2], F32)

        # All x loads up-front, dependency-free. W/bias ride the sync ring
        # right behind the first x segment: they complete in-ring long before
        # that segment's matmuls can start, without delaying the stream head.
        xt = {}
        for i, (name, objs, br) in enumerate(SEGS):
            t = xpools[name].tile(
                [128, len(objs), 2, br[1] - br[0]], BF16, name=f"t_{name}"
            )
            nc.sync.dma_start(t[:], xs[name][:])
            xt[name] = t
            if i == 0:
                nc.sync.dma_start(w_sb[:], wt[:])
                nc.sync.dma_start(b_sb[:], bt[:])

        # pair index -> (segment name, first-object index within segment)
        pair_seg = {0: ("xq0", 0), 1: ("xq0", 2), 2: ("xq1", 0), 3: ("xq1", 2),
                    4: ("xq2", 0), 5: ("xq2", 2), 6: ("xp6", 0)}

        ot = {}
        for op in range(7):  # pairs 0-6: full-batch compute
            g, pr = divmod(op, 2)
            if pr == 0:
                ot[g] = opool.tile([128, 2, BS], BF16, name=f"ot{g}")
            seg, j0 = pair_seg[op]
            for n in range(NB):
                ps = psum.tile([128, NT], F32)
                for o2 in range(2):
                    for k in range(2):
                        nc.tensor.matmul(
                            ps[o2 * 64 : (o2 + 1) * 64, :],
                            w_sb[:, k, 2 * op + o2, :],
                            xt[seg][:, j0 + o2, k, n * NT : (n + 1) * NT],
                            start=(k == 0),
                            stop=(k == 1),
                        )
                nc.scalar.activation(
                    ot[g][:, pr, n * NT : (n + 1) * NT],
                    ps[:],
                    mybir.ActivationFunctionType.Identity,
                    bias=b_sb[:, op : op + 1],
                )
            if pr == 1:
                nc.scalar.dma_start(out[g], ot[g])
            elif op == 6:
                nc.scalar.dma_start(out[3, :, 0, :], ot[3][:, 0, :])

        # pair 7: per-chunk compute from the batch-half tiles; stores on the
        # (now idle) sync queue so the scalar ACT chain isn't extended.
        for n in range(NB):
            seg = "xh0" if n < 2 else "xh1"
            b0 = (n % 2) * NT
            ps = psum.tile([128, NT], F32)
            for o2 in range(2):
                for k in range(2):
                    nc.tensor.matmul(
                        ps[o2 * 64 : (o2 + 1) * 64, :],
                        w_sb[:, k, 14 + o2, :],
                        xt[seg][:, o2, k, b0 : b0 + NT],
                        start=(k == 0),
                        stop=(k == 1),
                    )
            nc.scalar.activation(
                ot[3][:, 1, n * NT : (n + 1) * NT],
                ps[:],
                mybir.ActivationFunctionType.Identity,
                bias=b_sb[:, 7:8],
            )
            nc.sync.dma_start(
                out[3, :, 1, n * NT : (n + 1) * NT],
                ot[3][:, 1, n * NT : (n + 1) * NT],
            )

    nc.compile()
    return nc


def _get_nc():
    if "nc" not in _CACHE:
        _CACHE["nc"] = _build_nc()
    return _CACHE["nc"]


def _prep_inputs(x, W, b):
    x16 = np.asarray(x, dtype=np.float32).astype(NP_BF16)
    # wt[d_lo, k_chunk, o, apk]: W[o,a,p,d,k] -> [d,o,apk] -> [2,128,o,apk] -> [128,2,o,apk]
    wt = np.ascontiguousarray(
        np.asarray(W, dtype=np.float32)
        .astype(NP_BF16)
        .transpose(3, 0, 1, 2, 4)
        .reshape(2, 128, N_OBJ, APK)
        .transpose(1, 0, 2, 3)
    )
    # bt[o2*64+apk, pair]
    bt = np.ascontiguousarray(
        np.asarray(b, dtype=np.float32)
        .reshape(N_OBJ // 2, 2, APK)
        .transpose(1, 2, 0)
        .reshape(128, N_OBJ // 2)
    )
    in_maps = []
    for c in range(N_CORES):
        xc = x16[c * BS : (c + 1) * BS]  # [BS, 16, 256] bf16
        m = {"wt": wt, "bt": bt}
        for name, objs, (b0, b1) in SEGS:
            # seg[p, j, k, b] = xc[b0+b, objs[j], k*128 + p]
            a = xc[b0:b1, objs, :]                       # [nb, no, 256]
            a = a.transpose(2, 1, 0)                     # [256, no, nb]
            a = a.reshape(2, 128, len(objs), b1 - b0)    # [k, p, j, b]
            m[name] = np.ascontiguousarray(a.transpose(1, 2, 0, 3))
        in_maps.append(m)
    return in_maps


def kernel(x, W, b, _trace=False, **run_kwargs):
    nc = _get_nc()
    in_maps = _prep_inputs(x, W, b)
    res = run_bass_kernel_spmd(
        nc, in_maps, core_ids=list(range(N_CORES)), trace=_trace, **run_kwargs
    )
    _CACHE["last_results"] = res
    out = np.empty((B, N_OBJ, APK), dtype=np.float32)
    for c in range(N_CORES):
        # arr[g, o2*64+apk, pr, b] -> [b, 4g+2pr+o2, apk]
        a = res.results[c]["out"].astype(np.float32)
        a = a.reshape(4, 2, APK, 2, BS).transpose(4, 0, 3, 1, 2)
        out[c * BS : (c + 1) * BS] = a.reshape(BS, N_OBJ, APK)
    return out.reshape(B, N_OBJ, 4, 2, 8)


# revision 12
# speedup vs baseline: 1.1329x; 1.1070x over previous
"""ObjectDecoder kernel for Trainium2 (8 NeuronCores, data-parallel over batch).

Computes out[b, o, a, p, k] = sum_d x[b, o, d] * W[o, a, p, d, k] + bias[o, a, p, k]
  x: [16384, 16, 256] f32, W: [16, 4, 2, 256, 8] f32, b: [16, 4, 2, 8] f32
  out: [16384, 16, 4, 2, 8] f32

Per-core plan (batch shard of 2048 rows), bf16 data path (tolerance 2e-2 vs
bf16 error ~3e-3): halves HBM traffic and runs the PE at 1 cycle/row.

The kernel is wire-bound on the 16 shared DMA engines (~26 B/ns each, ~420
GB/s aggregate). Measured TRN2 behaviors this schedule works around:
  - W/bias ride FIRST on the sync ring: on any other queue they starve behind
    the x stream (engines round-robin rings per descriptor) and delay the
    first matmul by ~10us. In-ring FIFO lands them before any x data.
  - The Tile framework rotates ~8 DMA-completion semaphores; a reused
    semaphore adds a wait for the 8-back DMA's completion. With exactly 8
    loads (W, b, 6 x segments) ahead of the 7 stores, every store's reuse
    guard lands on an early load it transitively depends on anyway.
  - Each of the 4 output tiles is a distinct buffer so the ACT chain never
    waits on a store's (lagging) completion semaphore.
  - The last pair is loaded in batch-halves and stored per-chunk on the
    (by then idle) sync queue so the drain tail is short.

Layouts (host-prepped):
  xsN[p(128), obj_in_seg, k(2), b] with d = k*128 + p  (contraction on
    partitions; 16-32 KiB contiguous per partition line per load)
  wt[d_lo(128), k_chunk(2), obj(16), apk(64)], bt[(o2*64+apk)(128), pair(8)]
  out[group(4), (o2*64+apk)(128), pair_in_group(2), b]  (obj = 4g + 2pr + o2)

Per pair of objects: 4 matmuls [K=128, M=64, N=512] accumulate into a
[128, 512] PSUM bank (objects 2i / 2i+1 stacked on partitions); the scalar
engine evacuates PSUM with a fused per-partition bias add, converting to bf16.
"""

import os
from contextlib import ExitStack

os.environ.setdefault("JAX_PLATFORMS", "axon")

import ml_dtypes
import numpy as np

import concourse.bass as bass
import concourse.mybir as mybir
import concourse.tile as tile
from concourse import bacc
from concourse.bass_utils import run_bass_kernel_spmd

B, N_OBJ, DIM_IN, APK = 16384, 16, 256, 64
N_CORES = 8
BS = B // N_CORES          # 2048 batch rows per core
NT = 512                   # moving-operand tile (one PSUM bank in fp32)
NB = BS // NT              # 4 batch chunks per core
F32 = mybir.dt.float32
BF16 = mybir.dt.bfloat16
NP_BF16 = ml_dtypes.bfloat16

# x segments: (name, [objects], batch range) — quads first, then pair 6, then
# pair 7 in batch-halves so the drain tail is only half a pair deep.
SEGS = [
    ("xq0", [0, 1, 2, 3], (0, BS)),
    ("xq1", [4, 5, 6, 7], (0, BS)),
    ("xq2", [8, 9, 10, 11], (0, BS)),
    ("xp6", [12, 13], (0, BS)),
    ("xh0", [14, 15], (0, BS // 2)),
    ("xh1", [14, 15], (BS // 2, BS)),
]

_CACHE: dict = {}


def _build_nc():
    nc = bacc.Bacc("TRN2", target_bir_lowering=False, debug=False)

    xs = {
        name: nc.declare_dram_parameter(
            name, [128, len(objs), 2, br[1] - br[0]], BF16, isOutput=False
        )
        for name, objs, br in SEGS
    }
    wt = nc.declare_dram_parameter("wt", [128, 2, N_OBJ, APK], BF16, isOutput=False)
    bt = nc.declare_dram_parameter("bt", [128, N_OBJ // 2], F32, isOutput=False)
    out = nc.declare_dram_parameter("out", [4, 128, 2, BS], BF16, isOutput=True)

    with tile.TileContext(nc) as tc, ExitStack() as ctx:
        wpool = ctx.enter_context(tc.tile_pool(name="w", bufs=1))
        xpools = {
            name: ctx.enter_context(tc.tile_pool(name=name, bufs=1))
            for name, _, _ in SEGS
        }
        psum = ctx.enter_context(
            tc.tile_pool(name="ps", bufs=8, space=bass.MemorySpace.PSUM)
        )
        opool = ctx.enter_context(tc.tile_pool(name="o", bufs=1))

        # W + bias first on the sync ring.
        w_sb = wpool.tile([128, 2, N_OBJ, APK], BF16)
        nc.sync.dma_start(w_sb[:], wt[:])
        b_sb = wpool.tile([128, N_OBJ // 2], F32)
        nc.sync.dma_start(b_sb[:], bt[:])

        # All x loads up-front, dependency-free.
        xt = {}
        for name, objs, br in SEGS:
            t = xpools[name].tile(
                [128, len(objs), 2, br[1] - br[0]], BF16, name=f"t_{name}"
            )
            nc.sync.dma_start(t[:], xs[name][:])
            xt[name] = t

        # pair index -> (segment name, first-object index within segment)
        pair_seg = {0: ("xq0", 0), 1: ("xq0", 2), 2: ("xq1", 0), 3: ("xq1", 2),
                    4: ("xq2", 0), 5: ("xq2", 2), 6: ("xp6", 0)}

        ot = {}
        for op in range(7):  # pairs 0-6: full-batch compute
            g, pr = divmod(op, 2)
            if pr == 0:
                ot[g] = opool.tile([128, 2, BS], BF16, name=f"ot{g}")
            seg, j0 = pair_seg[op]
            for n in range(NB):
                ps = psum.tile([128, NT], F32)
                for o2 in range(2):
                    for k in range(2):
                        nc.tensor.matmul(
                            ps[o2 * 64 : (o2 + 1) * 64, :],
                            w_sb[:, k, 2 * op + o2, :],
                            xt[seg][:, j0 + o2, k, n * NT : (n + 1) * NT],
                            start=(k == 0),
                            stop=(k == 1),
                        )
                nc.scalar.activation(
                    ot[g][:, pr, n * NT : (n + 1) * NT],
                    ps[:],
                    mybir.ActivationFunctionType.Identity,
                    bias=b_sb[:, op : op + 1],
                )
            if pr == 1:
                nc.scalar.dma_start(out[g], ot[g])
            elif op == 6:
                nc.scalar.dma_start(out[3, :, 0, :], ot[3][:, 0, :])

        # pair 7: per-chunk compute from the batch-half tiles; stores on the
        # (now idle) sync queue so the scalar ACT chain isn't extended.
        for n in range(NB):
            seg = "xh0" if n < 2 else "xh1"
            b0 = (n % 2) * NT
            ps = psum.tile([128, NT], F32)
            for o2 in range(2):
                for k in range(2):
                    nc.tensor.matmul(
                        ps[o2 * 64 : (o2 + 1) * 64, :],
                        w_sb[:, k, 14 + o2, :],
                        xt[seg][:, o2, k, b0 : b0 + NT],
                        start=(k == 0),
                        stop=(k == 1),
                    )
            nc.scalar.activation(
                ot[3][:, 1, n * NT : (n + 1) * NT],
                ps[:],
                mybir.ActivationFunctionType.Identity,
                bias=b_sb[:, 7:8],
            )
            nc.sync.dma_start(
                out[3, :, 1, n * NT : (n + 1) * NT],
                ot[3][:, 1, n * NT : (n + 1) * NT],
            )

    nc.compile()
    return nc


def _get_nc():
    if "nc" not in _CACHE:
        _CACHE["nc"] = _build_nc()
    return _CACHE["nc"]


def _prep_inputs(x, W, b):
    x16 = np.asarray(x, dtype=np.float32).astype(NP_BF16)
    # wt[d_lo, k_chunk, o, apk]: W[o,a,p,d,k] -> [d,o,apk] -> [2,128,o,apk] -> [128,2,o,apk]
    wt = np.ascontiguousarray(
        np.asarray(W, dtype=np.float32)
        .astype(NP_BF16)
        .transpose(3, 0, 1, 2, 4)
        .reshape(2, 128, N_OBJ, APK)
        .transpose(1, 0, 2, 3)
    )
    # bt[o2*64+apk, pair]
    bt = np.ascontiguousarray(
        np.asarray(b, dtype=np.float32)
        .reshape(N_OBJ // 2, 2, APK)
        .transpose(1, 2, 0)
        .reshape(128, N_OBJ // 2)
    )
    in_maps = []
    for c in range(N_CORES):
        xc = x16[c * BS : (c + 1) * BS]  # [BS, 16, 256] bf16
        m = {"wt": wt, "bt": bt}
        for name, objs, (b0, b1) in SEGS:
            # seg[p, j, k, b] = xc[b0+b, objs[j], k*128 + p]
            a = xc[b0:b1, objs, :]                       # [nb, no, 256]
            a = a.transpose(2, 1, 0)                     # [256, no, nb]
            a = a.reshape(2, 128, len(objs), b1 - b0)    # [k, p, j, b]
            m[name] = np.ascontiguousarray(a.transpose(1, 2, 0, 3))
        in_maps.append(m)
    return in_maps


def kernel(x, W, b, _trace=False, **run_kwargs):
    nc = _get_nc()
    in_maps = _prep_inputs(x, W, b)
    res = run_bass_kernel_spmd(
        nc, in_maps, core_ids=list(range(N_CORES)), trace=_trace, **run_kwargs
    )
    _CACHE["last_results"] = res
    out = np.empty((B, N_OBJ, APK), dtype=np.float32)
    for c in range(N_CORES):
        # arr[g, o2*64+apk, pr, b] -> [b, 4g+2pr+o2, apk]
        a = res.results[c]["out"].astype(np.float32)
        a = a.reshape(4, 2, APK, 2, BS).transpose(4, 0, 3, 1, 2)
        out[c * BS : (c + 1) * BS] = a.reshape(BS, N_OBJ, APK)
    return out.reshape(B, N_OBJ, 4, 2, 8)
